# revision 5
# baseline (speedup 1.0000x reference)
"""MoE layer (8 experts, top-2, D=1024, FF=4096, N=4096 tokens) on 8 TRN2 cores.

Strategy: expert parallelism. Host computes the (cheap) router: logits,
top-2 + softmax weights, aux loss. Tokens are dispatched per expert on the
host; core e runs expert e's FFN  y = gelu(x @ w1[e]) @ w2[e]  on its routed
tokens (padded to fixed capacity C), activations kept transposed so both
weight matrices stay in natural layout as the stationary matmul operand.
Host applies the combine weights and scatter-adds the two expert outputs
per token.
"""

import numpy as np
import ml_dtypes

import concourse.bass as bass
import concourse.bacc as bacc
import concourse.mybir as mybir
import concourse.tile as tile
from concourse.bass_utils import run_bass_kernel_spmd

D_MODEL = 1024
N_EXPERTS = 8
TOP_K = 2
D_FF = 4096
N_TOKENS = 4096

# Token capacity per expert (compile-time). Actual max load for the
# fixed seed-0 inputs is 1066; 1280 leaves margin for fp/routing jitter.
CAPACITY = 1280
_CHUNKS = [512, 512, 256]  # sum == CAPACITY, each <= 512 (one PSUM bank)

BF16 = mybir.dt.bfloat16
F32 = mybir.dt.float32

_P = 128
_KD = D_MODEL // _P  # 8 k-tiles over d_model
_NJ = D_FF // _P  # 32 ff tiles
_NI = D_MODEL // _P  # 8 out d tiles


def _build_expert_ffn():
    """One expert's FFN: yT = (gelu(x @ w1) @ w2)^T for C tokens.

    Inputs:  xT (D, C) bf16, w1 (D, FF) bf16, w2 (FF, D) bf16
    Output:  yT (D, C) f32
    """
    nc = bacc.Bacc("TRN2", target_bir_lowering=False, debug=False)
    xT_d = nc.declare_dram_parameter("xT", [D_MODEL, CAPACITY], BF16, isOutput=False)
    w1_d = nc.declare_dram_parameter("w1", [D_MODEL, D_FF], BF16, isOutput=False)
    w2_d = nc.declare_dram_parameter("w2", [D_FF, D_MODEL], BF16, isOutput=False)
    yT_d = nc.declare_dram_parameter("yT", [D_MODEL, CAPACITY], F32, isOutput=True)

    with tile.TileContext(nc) as tc:
        with (
            tc.tile_pool(name="wts", bufs=1) as wts,
            tc.tile_pool(name="xsb", bufs=1) as xsb,
            tc.tile_pool(name="hsb", bufs=1) as hsb,
            tc.tile_pool(name="ysb", bufs=3) as ysb,
            tc.tile_pool(name="ps1", bufs=4, space="PSUM") as ps1,
            tc.tile_pool(name="ps2", bufs=3, space="PSUM") as ps2,
        ):
            # resident weights: w1 as 8 slabs (128, 4096), w2 as 32 slabs (128, 1024)
            w1_sb = [wts.tile([_P, D_FF], BF16, tag=f"w1_{k}", name=f"w1_{k}") for k in range(_KD)]
            for k in range(_KD):
                nc.sync.dma_start(w1_sb[k][:], w1_d[k * _P : (k + 1) * _P, :])
            # resident xT: 8 tiles (128, C)
            xT_sb = [xsb.tile([_P, CAPACITY], BF16, tag=f"xT_{k}", name=f"xT_{k}") for k in range(_KD)]
            for k in range(_KD):
                nc.sync.dma_start(xT_sb[k][:], xT_d[k * _P : (k + 1) * _P, :])
            w2_sb = [wts.tile([_P, D_MODEL], BF16, tag=f"w2_{j}", name=f"w2_{j}") for j in range(_NJ)]
            for j in range(_NJ):
                nc.sync.dma_start(w2_sb[j][:], w2_d[j * _P : (j + 1) * _P, :])

            c0 = 0
            for csz in _CHUNKS:
                cs = slice(c0, c0 + csz)
                c0 += csz
                # ---- layer 1: hT[j] = gelu(w1[:,j].T @ xT) ----
                hT = [hsb.tile([_P, csz], BF16, tag=f"hT_{j}", name=f"hT_{j}") for j in range(_NJ)]
                for j in range(_NJ):
                    acc = ps1.tile([_P, csz], F32, tag="l1acc")
                    for k in range(_KD):
                        nc.tensor.matmul(
                            acc[:],
                            w1_sb[k][:, j * _P : (j + 1) * _P],
                            xT_sb[k][:, cs],
                            start=(k == 0),
                            stop=(k == _KD - 1),
                        )
                    nc.scalar.activation(
                        hT[j][:], acc[:], mybir.ActivationFunctionType.Gelu
                    )
                # ---- layer 2: yT[i] = w2[:,i].T @ hT ----
                for i in range(_NI):
                    acc2 = ps2.tile([_P, csz], F32, tag="l2acc")
                    for j in range(_NJ):
                        nc.tensor.matmul(
                            acc2[:],
                            w2_sb[j][:, i * _P : (i + 1) * _P],
                            hT[j][:],
                            start=(j == 0),
                            stop=(j == _NJ - 1),
                        )
                    yt = ysb.tile([_P, csz], F32, tag="yt")
                    nc.vector.tensor_copy(yt[:], acc2[:])
                    nc.sync.dma_start(yT_d[i * _P : (i + 1) * _P, cs], yt[:])
    nc.compile()
    return nc


_NC_CACHE = None


def _get_nc():
    global _NC_CACHE
    if _NC_CACHE is None:
        _NC_CACHE = _build_expert_ffn()
    return _NC_CACHE


def kernel(x, gate_w, w1, w2):
    x = np.asarray(x, dtype=np.float32)
    gate_w = np.asarray(gate_w, dtype=np.float32)
    w1 = np.asarray(w1, dtype=np.float32)
    w2 = np.asarray(w2, dtype=np.float32)
    Bb, Tt, D = x.shape
    N = Bb * Tt
    flat_x = x.reshape(N, D)

    # ---- router (host) ----
    logits = flat_x @ gate_w  # (N, E) f32
    i1 = logits.argmax(axis=1)
    masked = logits.copy()
    masked[np.arange(N), i1] = -np.inf
    i2 = masked.argmax(axis=1)
    v1 = logits[np.arange(N), i1]
    v2 = logits[np.arange(N), i2]
    # softmax over the top-2 values (v1 >= v2)
    e2 = np.exp(v2 - v1)
    wt1 = 1.0 / (1.0 + e2)
    wt2 = e2 / (1.0 + e2)

    # aux loss, same math as reference (fp32)
    lmax = logits.max(axis=1, keepdims=True)
    p = np.exp(logits - lmax)
    p /= p.sum(axis=1, keepdims=True)
    m = p.mean(axis=0)
    aux_loss = np.float32((m * m).sum() * np.float32(N_EXPERTS))

    # ---- dispatch (host) ----
    tok_idx = []  # per expert: token ids
    tok_wt = []  # per expert: combine weights
    for e in range(N_EXPERTS):
        ids = np.concatenate([np.nonzero(i1 == e)[0], np.nonzero(i2 == e)[0]])
        wts = np.concatenate([wt1[i1 == e], wt2[i2 == e]])
        if len(ids) > CAPACITY:  # overflow guard; keep highest-weight tokens
            order = np.argsort(-wts)[:CAPACITY]
            ids, wts = ids[order], wts[order]
        tok_idx.append(ids)
        tok_wt.append(wts.astype(np.float32))

    in_maps = []
    for e in range(N_EXPERTS):
        ids = tok_idx[e]
        xT = np.zeros((D_MODEL, CAPACITY), dtype=ml_dtypes.bfloat16)
        xT[:, : len(ids)] = flat_x[ids].T.astype(ml_dtypes.bfloat16)
        in_maps.append(
            {
                "xT": xT,
                "w1": np.ascontiguousarray(w1[e]).astype(ml_dtypes.bfloat16),
                "w2": np.ascontiguousarray(w2[e]).astype(ml_dtypes.bfloat16),
            }
        )

    res = run_bass_kernel_spmd(_get_nc(), in_maps, core_ids=list(range(N_EXPERTS)))

    # ---- combine (host) ----
    out = np.zeros((N, D), dtype=np.float32)
    for e in range(N_EXPERTS):
        ids = tok_idx[e]
        y = res.results[e]["yT"][:, : len(ids)].T  # (n_e, D) f32
        out[ids] += tok_wt[e][:, None] * y
    return out.reshape(Bb, Tt, D), aux_loss


# revision 6
# speedup vs baseline: 1.1587x; 1.1587x over previous
"""MoE layer (8 experts, top-2, D=1024, FF=4096, N=4096 tokens) on 8 TRN2 cores.

Strategy: expert parallelism. Host computes the (cheap) router: logits,
top-2 + softmax weights, aux loss. Tokens are dispatched per expert on the
host; core e runs expert e's FFN  y = gelu(x @ w1[e]) @ w2[e]  on its routed
tokens (padded to fixed capacity C), activations kept transposed so both
weight matrices stay in natural layout as the stationary matmul operand.
Host applies the combine weights and scatter-adds the two expert outputs
per token.
"""

import numpy as np
import ml_dtypes

import concourse.bass as bass
import concourse.bacc as bacc
import concourse.mybir as mybir
import concourse.tile as tile
from concourse.bass_utils import run_bass_kernel_spmd

D_MODEL = 1024
N_EXPERTS = 8
TOP_K = 2
D_FF = 4096
N_TOKENS = 4096

# Token capacity per expert is chosen per call: the exact max expert load
# (the NEFF is compiled on first use for that capacity and cached).

BF16 = mybir.dt.bfloat16
F32 = mybir.dt.float32

_P = 128
_KD = D_MODEL // _P  # 8 k-tiles over d_model
_NJ = D_FF // _P  # 32 ff tiles
_NI = D_MODEL // _P  # 8 out d tiles


def _build_expert_ffn(capacity):
    """One expert's FFN: yT = (gelu(x @ w1) @ w2)^T for `capacity` tokens.

    Inputs:  xT (D, C) bf16, w1 (D, FF) bf16, w2 (FF, D) bf16
    Output:  yT (D, C) f32
    """
    chunks = [512] * (capacity // 512)
    if capacity % 512:
        chunks.append(capacity % 512)
    nc = bacc.Bacc("TRN2", target_bir_lowering=False, debug=False)
    xT_d = nc.declare_dram_parameter("xT", [D_MODEL, capacity], BF16, isOutput=False)
    w1_d = nc.declare_dram_parameter("w1", [D_MODEL, D_FF], BF16, isOutput=False)
    w2_d = nc.declare_dram_parameter("w2", [D_FF, D_MODEL], BF16, isOutput=False)
    yT_d = nc.declare_dram_parameter("yT", [D_MODEL, capacity], F32, isOutput=True)

    with tile.TileContext(nc) as tc:
        with (
            tc.tile_pool(name="wts", bufs=1) as wts,
            tc.tile_pool(name="xsb", bufs=1) as xsb,
            tc.tile_pool(name="hsb", bufs=1) as hsb,
            tc.tile_pool(name="ysb", bufs=3) as ysb,
            tc.tile_pool(name="ps1", bufs=4, space="PSUM") as ps1,
            tc.tile_pool(name="ps2", bufs=3, space="PSUM") as ps2,
        ):
            # resident weights: w1 as 8 slabs (128, 4096), w2 as 32 slabs (128, 1024)
            w1_sb = [wts.tile([_P, D_FF], BF16, tag=f"w1_{k}", name=f"w1_{k}") for k in range(_KD)]
            for k in range(_KD):
                nc.sync.dma_start(w1_sb[k][:], w1_d[k * _P : (k + 1) * _P, :])
            # resident xT: 8 tiles (128, C)
            xT_sb = [xsb.tile([_P, capacity], BF16, tag=f"xT_{k}", name=f"xT_{k}") for k in range(_KD)]
            for k in range(_KD):
                nc.sync.dma_start(xT_sb[k][:], xT_d[k * _P : (k + 1) * _P, :])
            w2_sb = [wts.tile([_P, D_MODEL], BF16, tag=f"w2_{j}", name=f"w2_{j}") for j in range(_NJ)]
            for j in range(_NJ):
                nc.sync.dma_start(w2_sb[j][:], w2_d[j * _P : (j + 1) * _P, :])

            c0 = 0
            for csz in chunks:
                cs = slice(c0, c0 + csz)
                c0 += csz
                # ---- layer 1: hT[j] = gelu(w1[:,j].T @ xT) ----
                hT = [hsb.tile([_P, csz], BF16, tag=f"hT_{j}", name=f"hT_{j}") for j in range(_NJ)]
                for j in range(_NJ):
                    acc = ps1.tile([_P, csz], F32, tag="l1acc")
                    for k in range(_KD):
                        nc.tensor.matmul(
                            acc[:],
                            w1_sb[k][:, j * _P : (j + 1) * _P],
                            xT_sb[k][:, cs],
                            start=(k == 0),
                            stop=(k == _KD - 1),
                        )
                    nc.scalar.activation(
                        hT[j][:], acc[:], mybir.ActivationFunctionType.Gelu
                    )
                # ---- layer 2: yT[i] = w2[:,i].T @ hT ----
                for i in range(_NI):
                    acc2 = ps2.tile([_P, csz], F32, tag="l2acc")
                    for j in range(_NJ):
                        nc.tensor.matmul(
                            acc2[:],
                            w2_sb[j][:, i * _P : (i + 1) * _P],
                            hT[j][:],
                            start=(j == 0),
                            stop=(j == _NJ - 1),
                        )
                    yt = ysb.tile([_P, csz], F32, tag="yt")
                    nc.vector.tensor_copy(yt[:], acc2[:])
                    nc.sync.dma_start(yT_d[i * _P : (i + 1) * _P, cs], yt[:])
    nc.compile()
    return nc


_NC_CACHE = {}


def _get_nc(capacity):
    if capacity not in _NC_CACHE:
        _NC_CACHE[capacity] = _build_expert_ffn(capacity)
    return _NC_CACHE[capacity]


def kernel(x, gate_w, w1, w2):
    x = np.asarray(x, dtype=np.float32)
    gate_w = np.asarray(gate_w, dtype=np.float32)
    w1 = np.asarray(w1, dtype=np.float32)
    w2 = np.asarray(w2, dtype=np.float32)
    Bb, Tt, D = x.shape
    N = Bb * Tt
    flat_x = x.reshape(N, D)

    # ---- router (host) ----
    logits = flat_x @ gate_w  # (N, E) f32
    i1 = logits.argmax(axis=1)
    masked = logits.copy()
    masked[np.arange(N), i1] = -np.inf
    i2 = masked.argmax(axis=1)
    v1 = logits[np.arange(N), i1]
    v2 = logits[np.arange(N), i2]
    # softmax over the top-2 values (v1 >= v2)
    e2 = np.exp(v2 - v1)
    wt1 = 1.0 / (1.0 + e2)
    wt2 = e2 / (1.0 + e2)

    # aux loss, same math as reference (fp32)
    lmax = logits.max(axis=1, keepdims=True)
    p = np.exp(logits - lmax)
    p /= p.sum(axis=1, keepdims=True)
    m = p.mean(axis=0)
    aux_loss = np.float32((m * m).sum() * np.float32(N_EXPERTS))

    # ---- dispatch (host) ----
    tok_idx = []  # per expert: token ids
    tok_wt = []  # per expert: combine weights
    for e in range(N_EXPERTS):
        ids = np.concatenate([np.nonzero(i1 == e)[0], np.nonzero(i2 == e)[0]])
        wts = np.concatenate([wt1[i1 == e], wt2[i2 == e]])
        tok_idx.append(ids)
        tok_wt.append(wts.astype(np.float32))

    capacity = max(512, max(len(ids) for ids in tok_idx))

    in_maps = []
    for e in range(N_EXPERTS):
        ids = tok_idx[e]
        xT = np.zeros((D_MODEL, capacity), dtype=ml_dtypes.bfloat16)
        xT[:, : len(ids)] = flat_x[ids].T.astype(ml_dtypes.bfloat16)
        in_maps.append(
            {
                "xT": xT,
                "w1": np.ascontiguousarray(w1[e]).astype(ml_dtypes.bfloat16),
                "w2": np.ascontiguousarray(w2[e]).astype(ml_dtypes.bfloat16),
            }
        )

    res = run_bass_kernel_spmd(_get_nc(capacity), in_maps, core_ids=list(range(N_EXPERTS)))

    # ---- combine (host) ----
    out = np.zeros((N, D), dtype=np.float32)
    for e in range(N_EXPERTS):
        ids = tok_idx[e]
        y = res.results[e]["yT"][:, : len(ids)].T  # (n_e, D) f32
        out[ids] += tok_wt[e][:, None] * y
    return out.reshape(Bb, Tt, D), aux_loss


# revision 11
# speedup vs baseline: 1.1732x; 1.0125x over previous
"""MoE layer (8 experts, top-2, D=1024, FF=4096, N=4096 tokens) on 8 TRN2 cores.

Strategy: expert parallelism. Host computes the (cheap) router: logits,
top-2 + softmax weights, aux loss. Tokens are dispatched per expert on the
host; core e runs expert e's FFN  y = gelu(x @ w1[e]) @ w2[e]  on its routed
tokens (padded to fixed capacity C), activations kept transposed so both
weight matrices stay in natural layout as the stationary matmul operand.
Host applies the combine weights and scatter-adds the two expert outputs
per token.
"""

import numpy as np
import ml_dtypes

import concourse.bass as bass
import concourse.bacc as bacc
import concourse.mybir as mybir
import concourse.tile as tile
from concourse.bass_utils import run_bass_kernel_spmd

D_MODEL = 1024
N_EXPERTS = 8
TOP_K = 2
D_FF = 4096
N_TOKENS = 4096

# Token capacity per expert is chosen per call: the exact max expert load
# (the NEFF is compiled on first use for that capacity and cached).

BF16 = mybir.dt.bfloat16
F32 = mybir.dt.float32

_P = 128
_KD = D_MODEL // _P  # 8 k-tiles over d_model
_NJ = D_FF // _P  # 32 ff tiles
_NI = D_MODEL // _P  # 8 out d tiles


def _build_expert_ffn(capacity):
    """One expert's FFN: yT = (gelu(x @ w1) @ w2)^T for `capacity` tokens.

    Inputs:  xT (D, C) bf16, w1 (D, FF) bf16, w2 (FF, D) bf16
    Output:  yT (D, C) f32
    """
    nch = -(-capacity // 512)
    base = capacity // nch
    chunks = [base + (1 if i < capacity - base * nch else 0) for i in range(nch)]
    nc = bacc.Bacc("TRN2", target_bir_lowering=False, debug=False)
    xT_d = nc.declare_dram_parameter("xT", [D_MODEL, capacity], BF16, isOutput=False)
    w1_d = nc.declare_dram_parameter("w1", [D_MODEL, D_FF], BF16, isOutput=False)
    w2_d = nc.declare_dram_parameter("w2", [D_FF, D_MODEL], BF16, isOutput=False)
    yT_d = nc.declare_dram_parameter("yT", [D_MODEL, capacity], F32, isOutput=True)

    with tile.TileContext(nc) as tc:
        with (
            tc.tile_pool(name="wts", bufs=1) as wts,
            tc.tile_pool(name="xsb", bufs=1) as xsb,
            tc.tile_pool(name="hsb", bufs=1) as hsb,
            tc.tile_pool(name="ysb", bufs=3) as ysb,
            tc.tile_pool(name="ps1", bufs=4, space="PSUM") as ps1,
            tc.tile_pool(name="ps2", bufs=3, space="PSUM") as ps2,
        ):
            # resident weights: w1 as 8 slabs (128, 4096), w2 as 32 slabs (128, 1024)
            w1_sb = [wts.tile([_P, D_FF], BF16, tag=f"w1_{k}", name=f"w1_{k}") for k in range(_KD)]
            for q in range(4):
                qs = slice(q * (D_FF // 4), (q + 1) * (D_FF // 4))
                for k in range(_KD):
                    nc.sync.dma_start(w1_sb[k][:, qs], w1_d[k * _P : (k + 1) * _P, qs])
            # resident xT: 8 tiles (128, C)
            xT_sb = [xsb.tile([_P, capacity], BF16, tag=f"xT_{k}", name=f"xT_{k}") for k in range(_KD)]
            cc = 0
            for csz in chunks:
                for k in range(_KD):
                    nc.sync.dma_start(
                        xT_sb[k][:, cc : cc + csz],
                        xT_d[k * _P : (k + 1) * _P, cc : cc + csz],
                    )
                cc += csz
            w2_sb = [wts.tile([_P, D_MODEL], BF16, tag=f"w2_{j}", name=f"w2_{j}") for j in range(_NJ)]
            for j in range(_NJ):
                nc.sync.dma_start(w2_sb[j][:], w2_d[j * _P : (j + 1) * _P, :])

            c0 = 0
            for csz in chunks:
                cs = slice(c0, c0 + csz)
                c0 += csz
                # ---- layer 1: hT[j] = gelu(w1[:,j].T @ xT) ----
                hT = [hsb.tile([_P, csz], BF16, tag=f"hT_{j}", name=f"hT_{j}") for j in range(_NJ)]
                for j in range(_NJ):
                    acc = ps1.tile([_P, csz], F32, tag="l1acc")
                    for k in range(_KD):
                        nc.tensor.matmul(
                            acc[:],
                            w1_sb[k][:, j * _P : (j + 1) * _P],
                            xT_sb[k][:, cs],
                            start=(k == 0),
                            stop=(k == _KD - 1),
                        )
                    nc.scalar.activation(
                        hT[j][:], acc[:], mybir.ActivationFunctionType.Gelu
                    )
                # ---- layer 2: yT[i] = w2[:,i].T @ hT ----
                for i in range(_NI):
                    acc2 = ps2.tile([_P, csz], F32, tag="l2acc")
                    for j in range(_NJ):
                        nc.tensor.matmul(
                            acc2[:],
                            w2_sb[j][:, i * _P : (i + 1) * _P],
                            hT[j][:],
                            start=(j == 0),
                            stop=(j == _NJ - 1),
                        )
                    yt = ysb.tile([_P, csz], F32, tag="yt")
                    nc.vector.tensor_copy(yt[:], acc2[:])
                    nc.sync.dma_start(yT_d[i * _P : (i + 1) * _P, cs], yt[:])
    nc.compile()
    return nc


_NC_CACHE = {}


def _get_nc(capacity):
    if capacity not in _NC_CACHE:
        _NC_CACHE[capacity] = _build_expert_ffn(capacity)
    return _NC_CACHE[capacity]


def kernel(x, gate_w, w1, w2):
    x = np.asarray(x, dtype=np.float32)
    gate_w = np.asarray(gate_w, dtype=np.float32)
    w1 = np.asarray(w1, dtype=np.float32)
    w2 = np.asarray(w2, dtype=np.float32)
    Bb, Tt, D = x.shape
    N = Bb * Tt
    flat_x = x.reshape(N, D)

    # ---- router (host) ----
    logits = flat_x @ gate_w  # (N, E) f32
    i1 = logits.argmax(axis=1)
    masked = logits.copy()
    masked[np.arange(N), i1] = -np.inf
    i2 = masked.argmax(axis=1)
    v1 = logits[np.arange(N), i1]
    v2 = logits[np.arange(N), i2]
    # softmax over the top-2 values (v1 >= v2)
    e2 = np.exp(v2 - v1)
    wt1 = 1.0 / (1.0 + e2)
    wt2 = e2 / (1.0 + e2)

    # aux loss, same math as reference (fp32)
    lmax = logits.max(axis=1, keepdims=True)
    p = np.exp(logits - lmax)
    p /= p.sum(axis=1, keepdims=True)
    m = p.mean(axis=0)
    aux_loss = np.float32((m * m).sum() * np.float32(N_EXPERTS))

    # ---- dispatch (host) ----
    tok_idx = []  # per expert: token ids
    tok_wt = []  # per expert: combine weights
    # SBUF budget bounds the resident xT tile; expected max load is ~1066
    # for these inputs, so the cap is far from binding. If it ever binds,
    # drop the lowest-weight tokens for that expert (graceful degradation).
    CAP_LIMIT = 1664
    for e in range(N_EXPERTS):
        ids = np.concatenate([np.nonzero(i1 == e)[0], np.nonzero(i2 == e)[0]])
        wts = np.concatenate([wt1[i1 == e], wt2[i2 == e]])
        if len(ids) > CAP_LIMIT:
            order = np.argsort(-wts)[:CAP_LIMIT]
            ids, wts = ids[order], wts[order]
        tok_idx.append(ids)
        tok_wt.append(wts.astype(np.float32))

    capacity = max(512, max(len(ids) for ids in tok_idx))

    in_maps = []
    for e in range(N_EXPERTS):
        ids = tok_idx[e]
        xT = np.zeros((D_MODEL, capacity), dtype=ml_dtypes.bfloat16)
        xT[:, : len(ids)] = flat_x[ids].T.astype(ml_dtypes.bfloat16)
        in_maps.append(
            {
                "xT": xT,
                "w1": np.ascontiguousarray(w1[e]).astype(ml_dtypes.bfloat16),
                "w2": np.ascontiguousarray(w2[e]).astype(ml_dtypes.bfloat16),
            }
        )

    res = run_bass_kernel_spmd(_get_nc(capacity), in_maps, core_ids=list(range(N_EXPERTS)))

    # ---- combine (host) ----
    out = np.zeros((N, D), dtype=np.float32)
    for e in range(N_EXPERTS):
        ids = tok_idx[e]
        y = res.results[e]["yT"][:, : len(ids)].T  # (n_e, D) f32
        out[ids] += tok_wt[e][:, None] * y
    return out.reshape(Bb, Tt, D), aux_loss


# revision 16
# speedup vs baseline: 1.2579x; 1.0722x over previous
"""MoE layer (8 experts, top-2, D=1024, FF=4096, N=4096 tokens) on 8 TRN2 cores.

Strategy: expert parallelism. Host computes the (cheap) router: logits,
top-2 + softmax weights, aux loss. Tokens are dispatched per expert on the
host; core e runs expert e's FFN  y = gelu(x @ w1[e]) @ w2[e]  on its routed
tokens (padded to fixed capacity C), activations kept transposed so both
weight matrices stay in natural layout as the stationary matmul operand.
Host applies the combine weights and scatter-adds the two expert outputs
per token.
"""

import numpy as np
import ml_dtypes

import concourse.bass as bass
import concourse.bacc as bacc
import concourse.mybir as mybir
import concourse.tile as tile
from concourse.bass_utils import run_bass_kernel_spmd

D_MODEL = 1024
N_EXPERTS = 8
TOP_K = 2
D_FF = 4096
N_TOKENS = 4096

# Token capacity per expert is chosen per call: the exact max expert load
# (the NEFF is compiled on first use for that capacity and cached).

BF16 = mybir.dt.bfloat16
F32 = mybir.dt.float32

_P = 128
_KD = D_MODEL // _P  # 8 k-tiles over d_model
_NJ = D_FF // _P  # 32 ff tiles
_NI = D_MODEL // _P  # 8 out d tiles


def _build_expert_ffn(capacity):
    nch = -(-capacity // 512)
    base = capacity // nch
    chunks = [base + (1 if i < capacity - base * nch else 0) for i in range(nch)]
    offs = [sum(chunks[:i]) for i in range(nch)]
    nc = bacc.Bacc("TRN2", target_bir_lowering=False, debug=False)
    xT_d = nc.declare_dram_parameter("xT", [D_MODEL, capacity], BF16, isOutput=False)
    w1_d = nc.declare_dram_parameter("w1", [D_MODEL, D_FF], BF16, isOutput=False)
    # host pre-arranges w2 as [i, p_ff, j, d]: each i-slab is one contiguous
    # 1MB block whose natural partition-major read fills w2i directly
    w2_d = nc.declare_dram_parameter("w2", [_NI, _P, _NJ, _P], BF16, isOutput=False)
    yT_d = nc.declare_dram_parameter("yT", [D_MODEL, capacity], F32, isOutput=True)

    with tile.TileContext(nc) as tc:
        with (
            tc.tile_pool(name="wts", bufs=1) as wts,
            tc.tile_pool(name="w2p", bufs=2) as w2p,
            tc.tile_pool(name="xsb", bufs=1) as xsb,
            tc.tile_pool(name="hsb", bufs=1) as hsb,
            tc.tile_pool(name="ysb", bufs=3) as ysb,
            tc.tile_pool(name="ps", bufs=2, space="PSUM") as ps,
        ):
            # resident w1 (8 slabs) + xT (8 slabs)
            w1_sb = [
                wts.tile([_P, D_FF], BF16, tag=f"w1_{k}", name=f"w1_{k}")
                for k in range(_KD)
            ]
            xT_sb = [
                xsb.tile([_P, capacity], BF16, tag=f"xT_{k}", name=f"xT_{k}")
                for k in range(_KD)
            ]
            # fine-grained first working set: per-k, first xT chunk + first
            # w1 column-eighth, interleaved, so j=0 unblocks after ~1 MB
            for k in range(_KD):
                nc.sync.dma_start(
                    xT_sb[k][:, : chunks[0]], xT_d[k * _P : (k + 1) * _P, : chunks[0]]
                )
                nc.sync.dma_start(
                    w1_sb[k][:, : D_FF // 8], w1_d[k * _P : (k + 1) * _P, : D_FF // 8]
                )
            for k in range(_KD):
                if capacity > chunks[0]:
                    nc.sync.dma_start(
                        xT_sb[k][:, chunks[0] :],
                        xT_d[k * _P : (k + 1) * _P, chunks[0] :],
                    )
            for q in range(1, 8):
                qs = slice(q * (D_FF // 8), (q + 1) * (D_FF // 8))
                for k in range(_KD):
                    nc.sync.dma_start(w1_sb[k][:, qs], w1_d[k * _P : (k + 1) * _P, qs])

            # hT fully resident: 32 slabs (128, capacity)
            hT = [
                hsb.tile([_P, capacity], BF16, tag=f"hT_{j}", name=f"hT_{j}")
                for j in range(_NJ)
            ]

            # ---- layer 1: chunk-innermost; LDW(w1[k,j]) shared by chunks ----
            for j in range(_NJ):
                accs = [
                    ps.tile([_P, csz], F32, tag=f"acc_{c}", name=f"acc_{c}")
                    for c, csz in enumerate(chunks)
                ]
                for k in range(_KD):
                    lhsT = w1_sb[k][:, j * _P : (j + 1) * _P]
                    for c, csz in enumerate(chunks):
                        nc.tensor.matmul(
                            accs[c][:],
                            lhsT,
                            xT_sb[k][:, offs[c] : offs[c] + csz],
                            start=(k == 0),
                            stop=(k == _KD - 1),
                        )
                for c, csz in enumerate(chunks):
                    nc.scalar.activation(
                        hT[j][:, offs[c] : offs[c] + csz],
                        accs[c][:],
                        mybir.ActivationFunctionType.Gelu,
                    )

            # ---- layer 2: w2 streamed per d-tile i; chunk-innermost ----
            for i in range(_NI):
                w2i = w2p.tile([_P, D_FF], BF16, tag="w2i", name="w2i")
                nc.sync.dma_start(w2i[:], w2_d[i])
                accs = [
                    ps.tile([_P, csz], F32, tag=f"acc_{c}", name=f"acc_{c}")
                    for c, csz in enumerate(chunks)
                ]
                for j in range(_NJ):
                    lhsT = w2i[:, j * _P : (j + 1) * _P]
                    for c, csz in enumerate(chunks):
                        nc.tensor.matmul(
                            accs[c][:],
                            lhsT,
                            hT[j][:, offs[c] : offs[c] + csz],
                            start=(j == 0),
                            stop=(j == _NJ - 1),
                        )
                for c, csz in enumerate(chunks):
                    yt = ysb.tile([_P, csz], F32, tag="yt", name="yt")
                    nc.vector.tensor_copy(yt[:], accs[c][:])
                    nc.sync.dma_start(
                        yT_d[i * _P : (i + 1) * _P, offs[c] : offs[c] + csz], yt[:]
                    )
    nc.compile()
    return nc




_NC_CACHE = {}


def _get_nc(capacity):
    if capacity not in _NC_CACHE:
        _NC_CACHE[capacity] = _build_expert_ffn(capacity)
    return _NC_CACHE[capacity]


def kernel(x, gate_w, w1, w2):
    x = np.asarray(x, dtype=np.float32)
    gate_w = np.asarray(gate_w, dtype=np.float32)
    w1 = np.asarray(w1, dtype=np.float32)
    w2 = np.asarray(w2, dtype=np.float32)
    Bb, Tt, D = x.shape
    N = Bb * Tt
    flat_x = x.reshape(N, D)

    # ---- router (host) ----
    logits = flat_x @ gate_w  # (N, E) f32
    i1 = logits.argmax(axis=1)
    masked = logits.copy()
    masked[np.arange(N), i1] = -np.inf
    i2 = masked.argmax(axis=1)
    v1 = logits[np.arange(N), i1]
    v2 = logits[np.arange(N), i2]
    # softmax over the top-2 values (v1 >= v2)
    e2 = np.exp(v2 - v1)
    wt1 = 1.0 / (1.0 + e2)
    wt2 = e2 / (1.0 + e2)

    # aux loss, same math as reference (fp32)
    lmax = logits.max(axis=1, keepdims=True)
    p = np.exp(logits - lmax)
    p /= p.sum(axis=1, keepdims=True)
    m = p.mean(axis=0)
    aux_loss = np.float32((m * m).sum() * np.float32(N_EXPERTS))

    # ---- dispatch (host) ----
    tok_idx = []  # per expert: token ids
    tok_wt = []  # per expert: combine weights
    # SBUF budget bounds the resident xT tile; expected max load is ~1066
    # for these inputs, so the cap is far from binding. If it ever binds,
    # drop the lowest-weight tokens for that expert (graceful degradation).
    CAP_LIMIT = 1664
    for e in range(N_EXPERTS):
        ids = np.concatenate([np.nonzero(i1 == e)[0], np.nonzero(i2 == e)[0]])
        wts = np.concatenate([wt1[i1 == e], wt2[i2 == e]])
        if len(ids) > CAP_LIMIT:
            order = np.argsort(-wts)[:CAP_LIMIT]
            ids, wts = ids[order], wts[order]
        tok_idx.append(ids)
        tok_wt.append(wts.astype(np.float32))

    capacity = max(512, max(len(ids) for ids in tok_idx))

    in_maps = []
    for e in range(N_EXPERTS):
        ids = tok_idx[e]
        xT = np.zeros((D_MODEL, capacity), dtype=ml_dtypes.bfloat16)
        xT[:, : len(ids)] = flat_x[ids].T.astype(ml_dtypes.bfloat16)
        in_maps.append(
            {
                "xT": xT,
                "w1": np.ascontiguousarray(w1[e]).astype(ml_dtypes.bfloat16),
                # [i, p_ff, j, d] so each output-d-tile slab is contiguous
                # and partition-major (matches the SBUF tile's natural AP)
                "w2": np.ascontiguousarray(
                    w2[e].reshape(32, 128, 8, 128).transpose(2, 1, 0, 3)
                ).astype(ml_dtypes.bfloat16),
            }
        )

    res = run_bass_kernel_spmd(_get_nc(capacity), in_maps, core_ids=list(range(N_EXPERTS)))

    # ---- combine (host) ----
    out = np.zeros((N, D), dtype=np.float32)
    for e in range(N_EXPERTS):
        ids = tok_idx[e]
        y = res.results[e]["yT"][:, : len(ids)].T  # (n_e, D) f32
        out[ids] += tok_wt[e][:, None] * y
    return out.reshape(Bb, Tt, D), aux_loss
